# revision 2
# baseline (speedup 1.0000x reference)
"""LocalizationAttacks kernel for 8 Trainium2 NeuronCores.

Data-parallel over the batch dim: each of the 8 cores processes 4 of the 32
batch items. Per-segment attack decisions (tiny [B, 300] masks) are
precomputed on the host and shipped as per-partition scalars; the 300 MB of
audio streaming (2 input streams, 3 output streams) runs on-device and is
fabric-bound at ~430 GB/s per core (measured: 2 HWDGE queues plateau at
425-438 GB/s; adding the gpsimd dynamic queue LOWERS aggregate to ~365; a
single queue alone sustains ~346-430). Floor = 38.4 MB / 430 GB/s ~= 89 us
+ ~7.2 us fixed Tile-framework preamble + ~4 us drain/epilogue.

Schedule: keep both HWDGE queues busy from ~8 us to ~97 us with no
availability stalls:
  - SYNC queue: all 12 input loads in tile order, then att/uo stores of
    tiles 0 and 5 (t0 is computed early, so sync's tail stores are ready
    the moment its loads finish; t5 is small so its data is ready right
    after the last load lands).
  - ACT queue: mask load first, then ground_truth stores (per-slice, ready
    almost immediately since gt depends only on the mask), then att/uo
    stores of tiles 1-4.
  - DVE: gt slices are store-paced (bufs=3), so t0-t3 att/uo compute is
    woven into the gaps between gt slices; t4/t5 computed when their loads
    land. gt is computed as (wm_dummy * 0) + mask_col so no ones tile is
    needed (the dummy read recycles already-loaded wm tiles).
Byte balance: sync 19.9 MB vs ACT 18.5 MB (ACT starts ~4 us later).
"""

import numpy as np

import concourse.bacc as bacc
import concourse.bass as bass
import concourse.mybir as mybir
from concourse.bass_utils import run_bass_kernel_spmd
from concourse.tile import TileContext

# Problem shape (hardcoded per contract)
B, C, T = 32, 1, 480000
SEG = 1600
S = T // SEG              # 300 segments per item
N_CORES = 8
B_LOC = B // N_CORES      # 4 items per core
N_SEGS = B_LOC * S        # 1200 segments per core
P = 128

# (partitions, segments-per-partition-row) per tile; rows sum to N_SEGS
PLAN = [(128, 2), (128, 2), (128, 2), (38, 2), (128, 2), (50, 2)]
assert sum(p * k for p, k in PLAN) == N_SEGS
N_MASK_COLS = 3 * sum(k for _, k in PLAN)
SYNC_TILES = (0, 5)  # att/uo of these tiles stored on the sync queue

F32 = mybir.dt.float32


def _build_nc() -> bass.Bass:
    nc = bacc.Bacc()
    wm = nc.dram_tensor("wm", [N_SEGS * SEG], F32, kind="ExternalInput")
    og = nc.dram_tensor("og", [N_SEGS * SEG], F32, kind="ExternalInput")
    mk = nc.dram_tensor("mk", [P, N_MASK_COLS], F32, kind="ExternalInput")
    att = nc.dram_tensor("att", [N_SEGS * SEG], F32, kind="ExternalOutput")
    gt = nc.dram_tensor("gt", [N_SEGS * SEG], F32, kind="ExternalOutput")
    uo = nc.dram_tensor("uo", [N_SEGS * SEG], F32, kind="ExternalOutput")

    mult = mybir.AluOpType.mult
    add = mybir.AluOpType.add

    def view(t, e0, p, k):
        return t[e0 : e0 + p * k * SEG].rearrange("(p f) -> p f", p=p)

    # per-tile (dram offset, mask column block offset)
    offs = []
    e0 = off = 0
    for p, k in PLAN:
        offs.append((e0, off))
        e0 += p * k * SEG
        off += k

    with TileContext(nc) as tc:
        with tc.tile_pool(name="io", bufs=2) as pool:
            pad = [P, 2 * SEG]
            m_all = pool.tile([P, N_MASK_COLS], F32, tag="m", bufs=1)
            nc.scalar.dma_start(out=m_all[:], in_=mk[:, :])

            # Pass 1: all loads on the sync queue, tile order.
            in_tiles = []
            for i, (p, k) in enumerate(PLAN):
                e0 = offs[i][0]
                wm_t = pool.tile([p, k * SEG], F32, tag="wm", bufs=2, padded_shape=pad)
                og_t = pool.tile([p, k * SEG], F32, tag="og", bufs=2, padded_shape=pad)
                nc.sync.dma_start(out=wm_t[:], in_=view(wm, e0, p, k))
                nc.sync.dma_start(out=og_t[:], in_=view(og, e0, p, k))
                in_tiles.append((wm_t, og_t))

            out_tiles = {}

            def emit_gt(i, j):
                p, k = PLAN[i]
                e0, off = offs[i]
                c = 3 * (off + j)
                # dummy input: a wm tile that is already resident (t0 uses
                # wm0; later tiles use wm1 so wm0's buffer can recycle).
                dummy = in_tiles[0 if i == 0 else 1][0]
                gts = pool.tile([p, SEG], F32, tag="gt", bufs=3,
                                padded_shape=[P, SEG])
                nc.vector.tensor_scalar(
                    gts[:], dummy[:p, :SEG], 0.0, m_all[:p, c : c + 1], mult, add
                )
                gv = view(gt, e0, p, k)[:, j * SEG : (j + 1) * SEG]
                nc.scalar.dma_start(out=gv, in_=gts[:])

            def compute_ops(i):
                """Return the list of DVE thunks for tile i's att/uo."""
                p, k = PLAN[i]
                off = offs[i][1]
                at_tag, uo_tag, nb = (
                    ("as", "us", 2) if i in SYNC_TILES else ("aa", "ua", 3)
                )
                at_t = pool.tile([p, k * SEG], F32, tag=at_tag, bufs=nb, padded_shape=pad)
                uo_t = pool.tile([p, k * SEG], F32, tag=uo_tag, bufs=nb, padded_shape=pad)
                out_tiles[i] = (at_t, uo_t)
                wm_t, og_t = in_tiles[i]
                ops = []
                for j in range(k):
                    sl = slice(j * SEG, (j + 1) * SEG)
                    c = 3 * (off + j)
                    s_am = m_all[:p, c + 0 : c + 1]  # 1 - attack
                    s_rm = m_all[:p, c + 1 : c + 2]  # revert
                    s_zm = m_all[:p, c + 2 : c + 3]  # 1 - zero
                    ops.append(lambda sl=sl, s=s_rm: nc.vector.tensor_scalar_mul(
                        at_t[:, sl], og_t[:, sl], s))
                    ops.append(lambda sl=sl, s=s_am: nc.vector.scalar_tensor_tensor(
                        at_t[:, sl], wm_t[:, sl], s, at_t[:, sl], mult, add))
                for j in range(k):
                    sl = slice(j * SEG, (j + 1) * SEG)
                    c = 3 * (off + j)
                    s_zm = m_all[:p, c + 2 : c + 3]
                    ops.append(lambda sl=sl, s=s_zm: nc.vector.tensor_scalar_mul(
                        uo_t[:, sl], og_t[:, sl], s))
                return ops

            def emit_store(i, ring):
                p, k = PLAN[i]
                e0 = offs[i][0]
                at_t, uo_t = out_tiles[i]
                ring.dma_start(out=view(att, e0, p, k), in_=at_t[:])
                ring.dma_start(out=view(uo, e0, p, k), in_=uo_t[:])

            # DVE weave: gt slices (store-paced) with t0-t3 att/uo compute
            # filling the gaps. Emission order on DVE == execution order.
            gt_slices = [(i, j) for i, (p, k) in enumerate(PLAN) for j in range(k)]
            comp = []
            for i in (0, 1, 2, 3):
                comp.extend(compute_ops(i))
            # prime ACT with the first two gt slices, then alternate
            emit_gt(*gt_slices[0])
            emit_gt(*gt_slices[1])
            ci = 0
            for s in range(2, len(gt_slices)):
                for _ in range(2):
                    if ci < len(comp):
                        comp[ci](); ci += 1
                emit_gt(*gt_slices[s])
            while ci < len(comp):
                comp[ci](); ci += 1

            # sync tail stores for t0 (ready long before sync's loads drain)
            emit_store(0, nc.sync)
            # ACT att/uo stores for t1-t3
            for i in (1, 2, 3):
                emit_store(i, nc.scalar)
            # t4: compute when its loads land, store on ACT
            for op in compute_ops(4):
                op()
            emit_store(4, nc.scalar)
            # t5 (small, last loads): compute, store on sync
            for op in compute_ops(5):
                op()
            emit_store(5, nc.sync)
    nc.compile()
    return nc


_NC_CACHE: bass.Bass | None = None


def _pack_masks(oma_rows, rm_rows, omz_rows):
    """Per-core segment masks [N_SEGS] -> one [P, N_MASK_COLS] tile."""
    m_all = np.zeros((P, N_MASK_COLS), np.float32)
    r0 = 0
    off = 0
    for p, k in PLAN:
        for j in range(k):
            c = 3 * (off + j)
            # partition q, slice j holds segment r0 + q*k + j
            m_all[:p, c + 0] = oma_rows[r0 + j : r0 + p * k : k]
            m_all[:p, c + 1] = rm_rows[r0 + j : r0 + p * k : k]
            m_all[:p, c + 2] = omz_rows[r0 + j : r0 + p * k : k]
        r0 += p * k
        off += k
    return m_all


def _prepare_in_maps(original, watermarked, seg_starts, revert_flags):
    original = np.ascontiguousarray(np.asarray(original), dtype=np.float32)
    watermarked = np.ascontiguousarray(np.asarray(watermarked), dtype=np.float32)
    seg_starts = np.asarray(seg_starts)
    revert_flags = np.asarray(revert_flags)

    # Host-side segment masks, [B, 300] each (tiny).
    attack = np.zeros((B, S), np.float32)
    attack[np.arange(B)[:, None], seg_starts] = 1.0
    rf = revert_flags.astype(np.float32)
    one_minus_am = 1.0 - attack
    rm = attack * rf
    one_minus_zm = 1.0 - attack * (1.0 - rf)

    in_maps = []
    for c in range(N_CORES):
        sl = slice(c * B_LOC, (c + 1) * B_LOC)
        in_maps.append(
            {
                "wm": watermarked[sl].reshape(-1),
                "og": original[sl].reshape(-1),
                "mk": _pack_masks(
                    one_minus_am[sl].reshape(-1),
                    rm[sl].reshape(-1),
                    one_minus_zm[sl].reshape(-1),
                ),
            }
        )
    return in_maps


def _gather(results):
    def cat(name):
        return np.concatenate(
            [results[c][name].reshape(B_LOC, C, T) for c in range(N_CORES)], axis=0
        )

    return cat("att"), cat("gt"), cat("uo")


def _run(inputs: dict, **run_kwargs):
    global _NC_CACHE
    if _NC_CACHE is None:
        _NC_CACHE = _build_nc()
    in_maps = _prepare_in_maps(**inputs)
    res = run_bass_kernel_spmd(
        _NC_CACHE, in_maps, core_ids=list(range(N_CORES)), **run_kwargs
    )
    return res, _gather(res.results)


def kernel(original, watermarked, seg_starts, revert_flags):
    _, outs = _run(
        dict(
            original=original,
            watermarked=watermarked,
            seg_starts=seg_starts,
            revert_flags=revert_flags,
        )
    )
    return outs
